# revision 23
# baseline (speedup 1.0000x reference)
"""Trainium2 Bass kernel for nn_DC_SpatialAttention (deformable-conv spatial attention).

Sharding: pure data-parallel over batch, 2 batch items per NeuronCore x 8 cores.
v2 pipeline (per batch item):
  A: stream x [256,16384] f32 in 16 chunks (2 HWDGE queues); per chunk
     mean row (PE f32r) + exp (ACT) + exp-sum row (PE bf16) into per-half
     group psum regions; per-half readout (ln-split chain) -> avmx + xcp.
  B (per half, pipelined right after its A group): D_k = dcn*(avg,mx)
     via PE -> psum -> dsb (bf16) -> ONE 4-level-AP scatter DMA into the
     pre-shifted plane canvas dp[item] (DRAM, 136x136 per k).
  Windows: dsh/dodd = contiguous 66*136-row slabs of dp (1 DMA per
     half/parity; rows wrap, all tent reads 4B-aligned).
  Patches: per (h,c,ky) contiguous 8570-elem runs of xcp -> conv matmuls
     use 3-level rhs APs (no im2col copy).
  C: offset conv (12 matmuls/chunk, FD=512) -> ACT relu/sigmoid FD=1024
     -> 25-op all-bf16 DVE tent -> masked val -> PE sum over k (bf16).
Item-1 A/B is generator-fed inside C0; item-1 windows reload in 3 row
bands and patches reload after C0's last conv (WAR-safe, tiny stall).
BatchNorm: per-core sums -> AllReduce -> affine+sigmoid -> y.

PSUM map (f32 cols x partitions):
  conv   parts 0-112, cols 0:3072   (g*1024+q*512; g=oy,ox,mask)
  mean   parts 0-15  (8g..), cols 3072:4096 (s*512)
  out    parts 16-31, cols 3072:4096
  lse    parts 32-47 (32+8g..), cols 3072:4096
  dvx    parts 64-112, slots 3072:3584 / 3584:4096
"""

import os
import numpy as np
import ml_dtypes

import concourse.bass as bass
import concourse.bacc as bacc
import concourse.mybir as mybir
import concourse.tile as tile
from concourse.bass_utils import run_bass_kernel_spmd

F32 = mybir.dt.float32
F32R = mybir.dt.float32r
BF16 = mybir.dt.bfloat16
F16 = mybir.dt.float16
I32 = mybir.dt.int32
AF = mybir.ActivationFunctionType
OP = mybir.AluOpType

# ---------------- problem constants (hardcoded) ----------------
B, C, H, W = 16, 256, 128, 128
HW = H * W
K2 = 49
BN_EPS = 1e-5
N_CORES = 8
BPC = B // N_CORES

LSE_T = 45.0
LSE_C = 153.0
LN2 = 0.6931471805599453

PW = H + 6                      # 134 padded xc width
XCP_N = PW * PW                 # 17956
DPW = H + 8                     # 136 plane width
DPN = DPW * DPW                 # 18496
WIN_R = 66
WIN_N = WIN_R * DPW             # 8976
PATN = 64 * PW                  # 8576 patch free width
NP = 113                        # tent partitions 0:49 + 64:113 (holes 49:64)

N_TOTAL = float(B * HW)


def _ap(t, off, pairs):
    return bass.AP(t, off, [list(p) for p in pairs])


def build_program(debug=False):
    nc = bacc.Bacc("TRN2", target_bir_lowering=False, debug=False,
                   num_devices=N_CORES)

    xs = nc.dram_tensor("xs", [BPC, C, HW], F32R, kind="ExternalInput")
    wc = nc.dram_tensor("wc", [98, 147], F16, kind="ExternalInput")
    bias_d = nc.dram_tensor("bias", [128, 3], F32, kind="ExternalInput")
    sd0_d = nc.dram_tensor("sd0", [48, 16 * K2], F16, kind="ExternalInput")
    selA_f_d = nc.dram_tensor("selA_f", [128, 256], F32R, kind="ExternalInput")
    selA_b_d = nc.dram_tensor("selA_b", [128, 256], BF16, kind="ExternalInput")
    selC_d = nc.dram_tensor("selC", [128, 128], F16, kind="ExternalInput")
    o16_d = nc.dram_tensor("o16", [16, 1], F32, kind="ExternalInput")
    on16_d = nc.dram_tensor("on16", [1, 16], F32, kind="ExternalInput")
    gb_d = nc.dram_tensor("gb", [1, 2], F32, kind="ExternalInput")
    cst_d = nc.dram_tensor("cst", [128, 1], F32, kind="ExternalInput")
    y_d = nc.dram_tensor("y", [BPC, HW], F32, kind="ExternalOutput")

    dp_dram = [nc.dram_tensor(f"dp_dram{i}", [K2 * DPN], F16)
               for i in range(2)]
    xcp_dram = nc.dram_tensor("xcp_dram", [2 * XCP_N], F16)
    cc_in = nc.dram_tensor("cc_in", [4], F32)
    cc_out = nc.dram_tensor("cc_out", [4], F32, addr_space="Shared")
    cc_win = nc.dram_tensor("cc_win", [4], F32)
    cc_wout = nc.dram_tensor("cc_wout", [4], F32, addr_space="Shared")

    PS = nc.alloc_psum_tensor("PS", [128, 4096], F32)

    with tile.TileContext(nc) as tc:
        dsh = nc.alloc_sbuf_tensor("dsh", [128, WIN_N], F16)
        dodd = nc.alloc_sbuf_tensor("dodd", [128, WIN_N], F16)
        dsb = nc.alloc_sbuf_tensor("dsb", [128, 8192], F16)
        patch = [nc.alloc_sbuf_tensor(f"patch{h}", [98, PATN], F16)
                 for h in range(2)]
        avmx = nc.alloc_sbuf_tensor("avmx", [48, 1024], F16)
        out_sb = nc.alloc_sbuf_tensor("out_sb", [16, 2 * 1024], F32)
        accs = nc.alloc_sbuf_tensor("accs", [16, 4], F32)
        bnt = nc.alloc_sbuf_tensor("bnt", [16, 16], F32)
        wsb = nc.alloc_sbuf_tensor("wsb", [98, 147], F16)
        bsb = nc.alloc_sbuf_tensor("bsb", [128, 3], F32)
        nbsb = nc.alloc_sbuf_tensor("nbsb", [128, 3], F32)
        sd0 = nc.alloc_sbuf_tensor("sd0_s", [48, 16 * K2], F16)
        selA_f = nc.alloc_sbuf_tensor("selA_f_s", [128, 256], F32R)
        selA_b = nc.alloc_sbuf_tensor("selA_b_s", [128, 256], BF16)
        selC = nc.alloc_sbuf_tensor("selC_s", [128, 128], F16)
        o16 = nc.alloc_sbuf_tensor("o16_s", [16, 1], F32)
        on16 = nc.alloc_sbuf_tensor("on16_s", [1, 16], F32)
        gbs = nc.alloc_sbuf_tensor("gbs", [1, 2], F32)
        cstsb = nc.alloc_sbuf_tensor("cst_s", [128, 1], F32)
        zt = nc.alloc_sbuf_tensor("zt", [128, 1024], F16)
        tb = [nc.alloc_sbuf_tensor(f"tb{i}", [128, 1024], F16)
              for i in range(9)]
        wgt = [nc.alloc_sbuf_tensor(f"wgt{i}", [128, 1024], F16)
               for i in range(8)]
        mkb = [nc.alloc_sbuf_tensor(f"mk{i}", [128, 1024], F16)
               for i in range(2)]
        bab = [nc.alloc_sbuf_tensor(f"ba{i}", [128, 1024], F16)
               for i in range(2)]

        dma = nc.sync.dma_start

        dma(wsb.ap(), wc.ap())
        dma(bsb.ap(), bias_d.ap())
        dma(sd0.ap(), sd0_d.ap())
        dma(selA_f.ap(), selA_f_d.ap())
        dma(selA_b.ap(), selA_b_d.ap())
        dma(selC.ap(), selC_d.ap())
        dma(o16.ap(), o16_d.ap())
        dma(on16.ap(), on16_d.ap())
        dma(gbs.ap(), gb_d.ap())
        dma(cstsb.ap(), cst_d.ap())
        nc.vector.tensor_scalar_mul(nbsb.ap(), bsb.ap(), -1.0)
        # warm up the collective path while A0 streams
        dma(cc_win.ap(), _ap(cstsb, 0, [[1, 4], [1, 1]]))
        nc.gpsimd.collective_compute(
            "AllReduce", OP.add,
            replica_groups=[list(range(N_CORES))],
            ins=[cc_win.ap()], outs=[cc_wout.ap()])

        # one-time zero inits
        nc.vector.memset(_ap(PS, 0, [[4096, 128], [1, 4096]]), 0.0)
        nc.gpsimd.memset(zt.ap(), 0.0)
        nc.gpsimd.memset(dsh.ap(), 0.0)
        nc.gpsimd.memset(dodd.ap(), 0.0)
        nc.gpsimd.memset(avmx.ap(), 0.0)
        ztf = _ap(zt, 0, [[1024, 128], [1, 1024]])
        per = 128 * 1024
        # zero padded-xc image + both dp plane canvases (borders stay zero)
        for t, n in [(xcp_dram, 2 * XCP_N),
                     (dp_dram[0], K2 * DPN), (dp_dram[1], K2 * DPN)]:
            nfull = n // per
            for i in range(nfull):
                nc.gpsimd.dma_start(
                    _ap(t, i * per, [[1024, 128], [1, 1024]]), ztf)
            rem = n - nfull * per
            if rem:
                fr = rem // 1024
                off = nfull * per
                if fr:
                    nc.gpsimd.dma_start(
                        _ap(t, off, [[1024, fr], [1, 1024]]),
                        _ap(zt, 0, [[1024, fr], [1, 1024]]))
                tail = rem - fr * 1024
                if tail:
                    nc.gpsimd.dma_start(
                        _ap(t, off + fr * 1024, [[tail, 1], [1, tail]]),
                        _ap(zt, 0, [[tail, 1], [1, tail]]))

        out_ps = _ap(PS, 2560, [[4096, 16], [1, 1024]])

        with (
            tc.tile_pool(name="xp", bufs=3) as xp,
            tc.tile_pool(name="ep", bufs=4) as ep,
            tc.tile_pool(name="st", bufs=1) as stp,
        ):
            def emit_patches(b, lo=0, hi=8570):
                # contiguous per-partition runs: 28 dmas x 7 parts
                qs = [nc.sync, nc.scalar, nc.gpsimd] if b == 0 else \
                    [nc.sync, nc.gpsimd]
                i = 0
                for h in range(2):
                    for cch in range(2):
                        for ky in range(7):
                            q = qs[i % len(qs)]
                            i += 1
                            q.dma_start(
                                _ap(patch[h], (cch * 49 + 7 * ky) * PATN + lo,
                                    [[PATN, 7], [1, hi - lo]]),
                                _ap(xcp_dram,
                                    cch * XCP_N + (64 * h + ky) * PW + lo,
                                    [[1, 7], [1, hi - lo]]))

            def emit_windows(b, r0, r1, dq=None):
                # contiguous slab reads of dp: rows [r0,r1) full 136 width
                ln = (r1 - r0) * DPW
                for h in range(2):
                    q1 = dq or nc.sync
                    q2 = dq or nc.gpsimd
                    q1.dma_start(
                        _ap(dsh, 64 * h * WIN_N + r0 * DPW,
                            [[WIN_N, 49], [1, ln]]),
                        _ap(dp_dram[b], (64 * h + 3 + r0) * DPW,
                            [[DPN, 49], [1, ln]]))
                    q2.dma_start(
                        _ap(dodd, 64 * h * WIN_N + r0 * DPW,
                            [[WIN_N, 49], [1, ln]]),
                        _ap(dp_dram[b], (64 * h + 3 + r0) * DPW + 1,
                            [[DPN, 49], [1, ln]]))

            def phase_A_gen(b, skip_patches=False):
                VE = nc.vector
                for g in range(2):
                    for nl in range(8):
                        n = 8 * g + nl
                        xt = xp.tile([128, 2048], F32R, tag="xt")
                        dma(_ap(xt.tensor, xt.offset,
                                [[2048, 128], [1024, 2], [1, 1024]]),
                            _ap(xs, b * C * HW + n * 1024,
                                [[HW, 128], [128 * HW, 2], [1, 1024]]))
                        et = ep.tile([128, 2048], BF16, tag="et")
                        nc.scalar.activation(et[:, :], xt[:, :].bitcast(F32),
                                             AF.Exp, bias=cstsb.ap(),
                                             scale=LSE_T)
                        sfa = _ap(selA_f, 16 * n, [[256, 128], [1, 16]])
                        sba = _ap(selA_b, 16 * n, [[256, 128], [1, 16]])
                        # mean matmuls first: they need only xt, so the PE
                        # starts before the exp finishes (lse mms after)
                        for s in range(2):
                            for cb in range(2):
                                sl = slice(cb * 1024 + s * 512,
                                           cb * 1024 + (s + 1) * 512)
                                nc.tensor.matmul(
                                    _ap(PS, 1536 + s * 512,
                                        [[4096, 16], [1, 512]]),
                                    sfa, xt[:, sl],
                                    start=(nl == 0 and cb == 0),
                                    stop=(nl == 7 and cb == 1))
                        for s in range(2):
                            for cb in range(2):
                                sl = slice(cb * 1024 + s * 512,
                                           cb * 1024 + (s + 1) * 512)
                                nc.tensor.matmul(
                                    _ap(PS, 32 * 4096 + 1536 + s * 512,
                                        [[4096, 16], [1, 512]]),
                                    sba, et[:, sl],
                                    start=(nl == 0 and cb == 0),
                                    stop=(nl == 7 and cb == 1))
                        yield
                    # ---- group readout (full 16-row blocks; the other
                    # group's rows are stale/zero and never consumed) ----
                    nc.scalar.copy(
                        _ap(avmx, 0, [[1024, 16], [1, 1024]]),
                        _ap(PS, 1536, [[4096, 16], [1, 1024]]))
                    nc.gpsimd.dma_start(
                        _ap(xcp_dram, 3 * PW + 3 + g * 64 * PW,
                            [[8 * PW, 8], [PW, 8], [1, 128]]),
                        _ap(avmx, 8 * g * 1024, [[1024, 8], [1, 1024]]))
                    # lse: ln(S) = Eraw*ln2 - 127*ln2 + ln(M), M in [1,2)
                    lse_ps = _ap(PS, 32 * 4096 + 1536, [[4096, 16], [1, 1024]])
                    bits = lse_ps.bitcast(I32)
                    ef_i = stp.tile([48, 1024], I32, tag="efi")
                    VE.tensor_scalar(ef_i[32:48, :], bits, 23, None,
                                     OP.arith_shift_right)
                    mf = stp.tile([48, 1024], F32, tag="mf")
                    VE.tensor_scalar(mf[32:48, :].bitcast(I32),
                                     bits, 0x007FFFFF, 0x3F800000,
                                     OP.bitwise_and, OP.bitwise_or)
                    ef = stp.tile([48, 1024], F32, tag="lnst")
                    VE.tensor_copy(ef[32:48, :], ef_i[32:48, :])
                    lnm = stp.tile([48, 1024], F32, tag="efi")
                    nc.scalar.activation(lnm[32:48, :], mf[32:48, :],
                                         AF.Ln)
                    nc.scalar.activation(mf[32:48, :], lnm[32:48, :],
                                         AF.Copy,
                                         bias=(LSE_C - 127.0 * LN2) / LSE_T,
                                         scale=1.0 / LSE_T)
                    VE.scalar_tensor_tensor(
                        _ap(avmx, 32 * 1024, [[1024, 16], [1, 1024]]),
                        ef[32:48, :], LN2 / LSE_T, mf[32:48, :],
                        OP.mult, OP.add)
                    nc.gpsimd.dma_start(
                        _ap(xcp_dram, XCP_N + 3 * PW + 3 + g * 64 * PW,
                            [[8 * PW, 8], [PW, 8], [1, 128]]),
                        _ap(avmx, (32 + 8 * g) * 1024, [[1024, 8], [1, 1024]]))
                    if g == 1 and not skip_patches:
                        emit_patches(b)
                    yield
                    # ---- B half g: D planes ----
                    for nl in range(8):
                        n = 8 * g + nl
                        for s in range(2):
                            slot = (2 * nl + s) % 2
                            dvx = _ap(PS, 64 * 4096 + 1536 + slot * 512,
                                      [[4096, 49], [1, 512]])
                            nc.tensor.matmul(
                                dvx,
                                _ap(sd0, n * K2, [[16 * K2, 48], [1, K2]]),
                                _ap(avmx, s * 512, [[1024, 48], [1, 512]]),
                                start=True, stop=True)
                            dst = _ap(dsb, 64 * 8192 + nl * 1024 + s * 512,
                                      [[8192, 49], [1, 512]])
                            if b == 0:
                                nc.vector.tensor_copy(dst, dvx)
                            else:
                                nc.scalar.copy(dst, dvx)
                        if nl % 2 == 1:
                            yield
                    # scatter half g: per-ky dmas, kx shift linear in
                    # partition (stride DPN-1), 256B rows
                    for ky in range(7):
                        nc.gpsimd.dma_start(
                            _ap(dp_dram[b],
                                7 * ky * DPN + (7 - ky + 64 * g) * DPW + 7,
                                [[DPN - 1, 7], [DPW, 64], [1, 128]]),
                            _ap(dsb, (64 + 7 * ky) * 8192,
                                [[8192, 7], [128, 64], [1, 128]]))
                    yield

            def sle(r0, i):
                return _ap(dsh, (r0 + 1 + i) * DPW + 4,
                           [[WIN_N, NP], [DPW, 8], [1, 128]])

            def slo(r0, i, j):
                return _ap(dodd, (r0 + 1 + i) * DPW + 3 + j,
                           [[WIN_N, NP], [DPW, 8], [1, 128]])

            def phase_C(b, feed=None, after_chunk=None, in_chunk7=None):
                for n in range(8):
                    r0 = 8 * n
                    wq = wgt[4 * (n % 2): 4 * (n % 2) + 4]
                    for q in range(2):
                        for h in range(2):
                            for g in range(3):
                                nc.tensor.matmul(
                                    _ap(PS, 64 * h * 4096 + g * 512,
                                        [[4096, 49], [1, 512]]),
                                    _ap(wsb, g * 49, [[147, 98], [1, 49]]),
                                    _ap(patch[h], (8 * n + 4 * q) * PW,
                                        [[PATN, 98], [PW, 4], [1, 128]]),
                                    start=True, stop=True)
                        if n == 7 and q == 1 and in_chunk7 is not None:
                            in_chunk7()
                        npv = lambda lo: _ap(PS, lo, [[4096, NP], [1, 512]])
                        wvq = lambda t: _ap(t, q * 512, [[1024, NP], [1, 512]])
                        # bilinear weights fused on ACT: relu(+-(conv+bias))
                        nc.scalar.activation(wvq(wq[0]), npv(0), AF.Relu,
                                             bias=nbsb.ap()[:NP, 0:1],
                                             scale=-1.0)
                        nc.scalar.activation(wvq(wq[1]), npv(0), AF.Relu,
                                             bias=bsb.ap()[:NP, 0:1])
                        nc.scalar.activation(wvq(wq[2]), npv(512), AF.Relu,
                                             bias=nbsb.ap()[:NP, 1:2],
                                             scale=-1.0)
                        nc.scalar.activation(wvq(wq[3]), npv(512), AF.Relu,
                                             bias=bsb.ap()[:NP, 1:2])
                        nc.scalar.activation(wvq(mkb[n % 2]), npv(1024),
                                             AF.Sigmoid,
                                             bias=bsb.ap()[:NP, 2:3])
                    if feed is not None:
                        for _ in range(5 if n < 6 else 0):
                            next(feed, None)
                    wv = lambda t: _ap(t, 0, [[1024, NP], [1, 1024]])
                    v = nc.vector
                    wym, wyp, wxm, wxp = (wv(w) for w in wq)
                    t = [wv(x) for x in tb]
                    D00 = sle(r0, 0)
                    v.tensor_sub(t[0], sle(r0, -1), D00)
                    v.tensor_sub(t[1], sle(r0, 1), D00)
                    v.tensor_mul(t[2], wym, t[0])
                    v.tensor_mul(t[3], wyp, t[1])
                    v.tensor_add(t[4], t[2], t[3])
                    v.tensor_add(t[5], D00, t[4])              # G0
                    v.tensor_sub(t[0], slo(r0, -1, -1), slo(r0, 0, -1))
                    v.tensor_sub(t[1], slo(r0, 1, -1), slo(r0, 0, -1))
                    v.tensor_mul(t[2], wym, t[0])
                    v.tensor_mul(t[3], wyp, t[1])
                    v.tensor_add(t[6], t[2], t[3])
                    v.tensor_add(t[7], slo(r0, 0, -1), t[6])   # Gm
                    v.tensor_sub(t[0], slo(r0, -1, 1), slo(r0, 0, 1))
                    v.tensor_sub(t[1], slo(r0, 1, 1), slo(r0, 0, 1))
                    v.tensor_mul(t[2], wym, t[0])
                    v.tensor_mul(t[3], wyp, t[1])
                    v.tensor_add(t[6], t[2], t[3])
                    v.tensor_add(t[8], slo(r0, 0, 1), t[6])    # Gp
                    v.tensor_sub(t[0], t[7], t[5])
                    v.tensor_sub(t[1], t[8], t[5])
                    v.tensor_mul(t[2], wxm, t[0])
                    v.tensor_mul(t[3], wxp, t[1])
                    v.tensor_add(t[6], t[2], t[3])
                    v.tensor_add(t[7], t[5], t[6])             # val
                    v.tensor_mul(wv(bab[n % 2]), wv(mkb[n % 2]), t[7])
                    scf = _ap(selC, 16 * n, [[128, NP], [1, 16]])
                    for s in range(2):
                        nc.tensor.matmul(
                            _ap(PS, 2560 + s * 512,
                                [[4096, 16], [1, 512]]),
                            scf,
                            _ap(bab[n % 2], s * 512, [[1024, NP], [1, 512]]),
                            start=(n == 0), stop=(n == 7))
                    if after_chunk is not None:
                        after_chunk(n)
                # item BN partial sums
                ob_v = _ap(out_sb, b * 1024, [[2048, 16], [1, 1024]])
                nc.scalar.copy(ob_v, out_ps)
                dump = stp.tile([128, 1024], F32, tag="mf")
                nc.scalar.activation(dump[0:16, :], ob_v, AF.Identity,
                                     accum_out=_ap(accs, 2 * b,
                                                   [[4, 16], [1, 1]]))
                nc.scalar.activation(dump[0:16, :], ob_v, AF.Square,
                                     accum_out=_ap(accs, 2 * b + 1,
                                                   [[4, 16], [1, 1]]))

            # ================= schedule =================
            a0 = phase_A_gen(0)
            for _ in a0:
                pass
            emit_windows(0, 0, WIN_R)

            a1 = phase_A_gen(1, skip_patches=True)

            def after_chunk(n):
                if n == 4:
                    emit_patches(1, 0, 4288)
                elif n == 5:
                    emit_windows(1, 0, 33, dq=nc.gpsimd)
                elif n == 6:
                    emit_windows(1, 33, 56, dq=nc.gpsimd)
                elif n == 7:
                    emit_windows(1, 56, WIN_R, dq=nc.gpsimd)
                # C0 chunk 5 runs after the full a1 drain (5 pulls x 6
                # chunks = 30 >= 28 yields), so scatter h1 precedes band0

            phase_C(0, feed=a1, after_chunk=after_chunk,
                    in_chunk7=lambda: emit_patches(1, 4288, 8570))
            for _ in a1:
                pass
            phase_C(1)

            # ---------- BN ----------
            bn_ps = _ap(PS, 3584, [[4096, 1], [1, 4]])
            nc.tensor.matmul(bn_ps, o16.ap(), accs.ap(),
                             start=True, stop=True)
            bnl = _ap(bnt, 0, [[16, 1], [1, 4]])
            nc.scalar.copy(bnl, bn_ps)
            dma(cc_in.ap(), bnl)
            nc.gpsimd.collective_compute(
                "AllReduce", OP.add,
                replica_groups=[list(range(N_CORES))],
                ins=[cc_in.ap()], outs=[cc_out.ap()])
            bnr = _ap(bnt, 4, [[16, 1], [1, 4]])
            dma(bnr, cc_out.ap())
            v = nc.vector
            e = lambda i: _ap(bnt, 4 + i, [[16, 1], [1, 1]])
            t = lambda i: _ap(bnt, 8 + i, [[16, 1], [1, 1]])
            v.tensor_add(t(0), e(0), e(2))                  # sum
            v.tensor_add(t(1), e(1), e(3))                  # sumsq
            v.tensor_scalar_mul(t(2), t(0), 1.0 / N_TOTAL)  # mean
            v.tensor_scalar_mul(t(3), t(1), 1.0 / N_TOTAL)  # E[x^2]
            v.tensor_mul(t(4), t(2), t(2))
            v.tensor_sub(t(5), t(3), t(4))                  # var
            v.tensor_scalar_add(t(5), t(5), BN_EPS)
            v.reciprocal(t(6), t(5))
            nc.scalar.sqrt(t(7), t(6))                      # rstd
            v.tensor_mul(_ap(bnt, 2, [[16, 1], [1, 1]]), t(7),
                         gbs.ap()[:, 0:1])                  # scale @ [0,2]
            v.tensor_mul(t(4), t(2), _ap(bnt, 2, [[16, 1], [1, 1]]))
            v.tensor_sub(_ap(bnt, 3, [[16, 1], [1, 1]]),
                         gbs.ap()[:, 1:2], t(4))            # bias @ [0,3]
            sb2 = _ap(bnt, 2, [[16, 1], [1, 2]])
            bcp = _ap(PS, 3600, [[4096, 16], [1, 2]])
            nc.tensor.matmul(bcp, on16.ap(), sb2, start=True, stop=True)
            nc.scalar.copy(_ap(bnt, 8, [[16, 16], [1, 2]]), bcp)
            for b in range(BPC):
                yb = stp.tile([128, 1024], F32, tag=f"yb{b}")
                nc.scalar.activation(yb[0:16, :],
                                     _ap(out_sb, b * 1024,
                                         [[2048, 16], [1, 1024]]),
                                     AF.Sigmoid,
                                     bias=_ap(bnt, 9, [[16, 16], [1, 1]]),
                                     scale=_ap(bnt, 8, [[16, 16], [1, 1]]))
                dma(_ap(y_d, b * HW, [[1024, 8], [8192, 2], [1, 1024]]),
                    yb[0:16, :])

    nc.compile()
    return nc


_NC_CACHE = None


def _get_nc():
    global _NC_CACHE
    if _NC_CACHE is None:
        _NC_CACHE = build_program()
    return _NC_CACHE


def make_host_constants(w_off, b_off, w_dcn, gamma, beta):
    bf = ml_dtypes.bfloat16
    orig = np.empty(147, np.int64)
    for g in range(3):
        for kk in range(49):
            orig[g * 49 + kk] = (2 * kk, 2 * kk + 1, 98 + kk)[g]
    wof = w_off.reshape(147, 2, 7, 7)
    wcl = np.zeros((98, 147), np.float32)
    for c in range(2):
        for ky in range(7):
            for kx in range(7):
                wcl[c * 49 + 7 * ky + kx, :] = wof[orig, c, ky, kx]
    # bias over partition convention p = 64*half + k  (holes zero)
    bias_t = np.zeros((128, 3), np.float32)
    for g in range(3):
        bg = b_off[orig[g * 49:(g + 1) * 49]]
        bias_t[0:49, g] = bg
        bias_t[64:113, g] = bg
    dcn = w_dcn.reshape(2, 49).astype(np.float32)
    sd0 = np.zeros((48, 16 * K2), np.float32)
    for n in range(16):
        sd0[n, 49 * n:49 * (n + 1)] = dcn[0]
        sd0[32 + n, 49 * n:49 * (n + 1)] = dcn[1]
    # phase-A row-spread selectors [128, 16*16]
    selA_f = np.zeros((128, 256), np.float32)
    selA_b = np.zeros((128, 256), np.float32)
    for n in range(16):
        selA_f[:, 16 * n + n] = 1.0 / C
        selA_b[:, 16 * n + n] = 1.0
    # phase-C sum-over-k selectors [128, 16*8]
    selC = np.zeros((128, 128), np.float32)
    for n in range(8):
        selC[0:49, 16 * n + 2 * n] = 1.0
        selC[64:113, 16 * n + 2 * n + 1] = 1.0
    return {
        "wc": wcl.astype(np.float16),
        "bias": bias_t,
        "sd0": sd0.astype(np.float16),
        "selA_f": selA_f,
        "selA_b": selA_b.astype(bf),
        "selC": selC.astype(np.float16),
        "o16": np.ones((16, 1), np.float32),
        "on16": np.ones((1, 16), np.float32),
        "gb": np.array([[float(np.reshape(gamma, -1)[0]),
                         float(np.reshape(beta, -1)[0])]], np.float32),
        "cst": np.full((128, 1), -LSE_C, np.float32),
    }


def make_in_maps(x, w_off, b_off, w_dcn, gamma, beta):
    consts = make_host_constants(w_off, b_off, w_dcn, gamma, beta)
    in_maps = []
    for i in range(N_CORES):
        m = dict(consts)
        m["xs"] = np.ascontiguousarray(
            x[i * BPC:(i + 1) * BPC].reshape(BPC, C, HW).astype(np.float32))
        in_maps.append(m)
    return in_maps


def kernel(x, w_off, b_off, w_dcn, gamma, beta):
    x = np.asarray(x, np.float32)
    nc = _get_nc()
    in_maps = make_in_maps(x, np.asarray(w_off, np.float32),
                           np.asarray(b_off, np.float32),
                           np.asarray(w_dcn, np.float32),
                           np.asarray(gamma, np.float32),
                           np.asarray(beta, np.float32))
    trace = bool(int(os.environ.get("KERNEL_TRACE", "0")))
    res = run_bass_kernel_spmd(nc, in_maps, core_ids=list(range(N_CORES)),
                               trace=trace)
    ys = [np.asarray(res.results[i]["y"], np.float32).reshape(BPC, HW)
          for i in range(N_CORES)]
    out = np.stack(ys).reshape(B, 1, H, W)
    kernel.last_exec_time_ns = res.exec_time_ns
    return out


# revision 24
# speedup vs baseline: 1.1006x; 1.1006x over previous
"""Trainium2 Bass kernel for nn_DC_SpatialAttention (deformable-conv spatial attention).

Sharding: pure data-parallel over batch, 2 batch items per NeuronCore x 8 cores.
v2 pipeline (per batch item):
  A: stream x [256,16384] f32 in 16 chunks (2 HWDGE queues); per chunk
     mean row (PE f32r) + exp (ACT) + exp-sum row (PE bf16) into per-half
     group psum regions; per-half readout (ln-split chain) -> avmx + xcp.
  B (per half, pipelined right after its A group): D_k = dcn*(avg,mx)
     via PE -> psum -> dsb (bf16) -> ONE 4-level-AP scatter DMA into the
     pre-shifted plane canvas dp[item] (DRAM, 136x136 per k).
  Windows: dsh/dodd = contiguous 66*136-row slabs of dp (1 DMA per
     half/parity; rows wrap, all tent reads 4B-aligned).
  Patches: per (h,c,ky) contiguous 8570-elem runs of xcp -> conv matmuls
     use 3-level rhs APs (no im2col copy).
  C: offset conv (12 matmuls/chunk, FD=512) -> ACT relu/sigmoid FD=1024
     -> 25-op all-bf16 DVE tent -> masked val -> PE sum over k (bf16).
Item-1 A/B is generator-fed inside C0; item-1 windows reload in 3 row
bands and patches reload after C0's last conv (WAR-safe, tiny stall).
BatchNorm: per-core sums -> AllReduce -> affine+sigmoid -> y.

PSUM map (f32 cols x partitions):
  conv   parts 0-112, cols 0:3072   (g*1024+q*512; g=oy,ox,mask)
  mean   parts 0-15  (8g..), cols 3072:4096 (s*512)
  out    parts 16-31, cols 3072:4096
  lse    parts 32-47 (32+8g..), cols 3072:4096
  dvx    parts 64-112, slots 3072:3584 / 3584:4096
"""

import os
import numpy as np
import ml_dtypes

import concourse.bass as bass
import concourse.bacc as bacc
import concourse.mybir as mybir
import concourse.tile as tile
from concourse.bass_utils import run_bass_kernel_spmd

F32 = mybir.dt.float32
F32R = mybir.dt.float32r
BF16 = mybir.dt.bfloat16
F16 = mybir.dt.float16
I32 = mybir.dt.int32
AF = mybir.ActivationFunctionType
OP = mybir.AluOpType

# ---------------- problem constants (hardcoded) ----------------
B, C, H, W = 16, 256, 128, 128
HW = H * W
K2 = 49
BN_EPS = 1e-5
N_CORES = 8
BPC = B // N_CORES

LSE_T = 45.0
LSE_C = 153.0
LN2 = 0.6931471805599453

PW = H + 6                      # 134 padded xc width
XCP_N = PW * PW                 # 17956
DPW = H + 8                     # 136 plane width
DPN = DPW * DPW                 # 18496
WIN_R = 66
WIN_N = WIN_R * DPW             # 8976
PATN = 64 * PW                  # 8576 patch free width
NP = 113                        # tent partitions 0:49 + 64:113 (holes 49:64)

N_TOTAL = float(B * HW)


def _ap(t, off, pairs):
    return bass.AP(t, off, [list(p) for p in pairs])


def build_program(debug=False):
    nc = bacc.Bacc("TRN2", target_bir_lowering=False, debug=False,
                   num_devices=N_CORES)

    xs = nc.dram_tensor("xs", [BPC, C, HW], F32R, kind="ExternalInput")
    wc = nc.dram_tensor("wc", [98, 147], F16, kind="ExternalInput")
    bias_d = nc.dram_tensor("bias", [128, 3], F32, kind="ExternalInput")
    sd0_d = nc.dram_tensor("sd0", [48, 16 * K2], F16, kind="ExternalInput")
    selA_f_d = nc.dram_tensor("selA_f", [128, 256], F32R, kind="ExternalInput")
    selA_b_d = nc.dram_tensor("selA_b", [128, 256], BF16, kind="ExternalInput")
    selC_d = nc.dram_tensor("selC", [128, 128], F16, kind="ExternalInput")
    o16_d = nc.dram_tensor("o16", [16, 1], F32, kind="ExternalInput")
    on16_d = nc.dram_tensor("on16", [1, 16], F32, kind="ExternalInput")
    gb_d = nc.dram_tensor("gb", [1, 2], F32, kind="ExternalInput")
    cst_d = nc.dram_tensor("cst", [128, 1], F32, kind="ExternalInput")
    y_d = nc.dram_tensor("y", [BPC, HW], F32, kind="ExternalOutput")

    dp_dram = [nc.dram_tensor(f"dp_dram{i}", [K2 * DPN], F16)
               for i in range(2)]
    xcp_dram = nc.dram_tensor("xcp_dram", [2 * XCP_N], F16)
    cc_in = nc.dram_tensor("cc_in", [4], F32)
    cc_out = nc.dram_tensor("cc_out", [4], F32, addr_space="Shared")
    cc_win = nc.dram_tensor("cc_win", [4], F32)
    cc_wout = nc.dram_tensor("cc_wout", [4], F32, addr_space="Shared")

    PS = nc.alloc_psum_tensor("PS", [128, 4096], F32)

    with tile.TileContext(nc) as tc:
        dsh = nc.alloc_sbuf_tensor("dsh", [128, WIN_N], F16)
        dodd = nc.alloc_sbuf_tensor("dodd", [128, WIN_N], F16)
        dsb = nc.alloc_sbuf_tensor("dsb", [128, 8192], F16)
        patch = [nc.alloc_sbuf_tensor(f"patch{h}", [98, PATN], F16)
                 for h in range(2)]
        avmx = nc.alloc_sbuf_tensor("avmx", [48, 1024], F16)
        out_sb = nc.alloc_sbuf_tensor("out_sb", [16, 2 * 1024], F32)
        accs = nc.alloc_sbuf_tensor("accs", [16, 4], F32)
        bnt = nc.alloc_sbuf_tensor("bnt", [16, 16], F32)
        wsb = nc.alloc_sbuf_tensor("wsb", [98, 147], F16)
        bsb = nc.alloc_sbuf_tensor("bsb", [128, 3], F32)
        nbsb = nc.alloc_sbuf_tensor("nbsb", [128, 3], F32)
        sd0 = nc.alloc_sbuf_tensor("sd0_s", [48, 16 * K2], F16)
        selA_f = nc.alloc_sbuf_tensor("selA_f_s", [128, 256], F32R)
        selA_b = nc.alloc_sbuf_tensor("selA_b_s", [128, 256], BF16)
        selC = nc.alloc_sbuf_tensor("selC_s", [128, 128], F16)
        o16 = nc.alloc_sbuf_tensor("o16_s", [16, 1], F32)
        on16 = nc.alloc_sbuf_tensor("on16_s", [1, 16], F32)
        gbs = nc.alloc_sbuf_tensor("gbs", [1, 2], F32)
        cstsb = nc.alloc_sbuf_tensor("cst_s", [128, 1], F32)
        zt = nc.alloc_sbuf_tensor("zt", [128, 1024], F16)
        tb = [nc.alloc_sbuf_tensor(f"tb{i}", [128, 1024], F16)
              for i in range(9)]
        wgt = [nc.alloc_sbuf_tensor(f"wgt{i}", [128, 1024], F16)
               for i in range(8)]
        mkb = [nc.alloc_sbuf_tensor(f"mk{i}", [128, 1024], F16)
               for i in range(2)]
        bab = [nc.alloc_sbuf_tensor(f"ba{i}", [128, 1024], F16)
               for i in range(2)]

        dma = nc.sync.dma_start

        dma(wsb.ap(), wc.ap())
        dma(bsb.ap(), bias_d.ap())
        dma(sd0.ap(), sd0_d.ap())
        dma(selA_f.ap(), selA_f_d.ap())
        dma(selA_b.ap(), selA_b_d.ap())
        dma(selC.ap(), selC_d.ap())
        dma(o16.ap(), o16_d.ap())
        dma(on16.ap(), on16_d.ap())
        dma(gbs.ap(), gb_d.ap())
        dma(cstsb.ap(), cst_d.ap())
        nc.vector.tensor_scalar_mul(nbsb.ap(), bsb.ap(), -1.0)
        # warm up the collective path while A0 streams
        dma(cc_win.ap(), _ap(cstsb, 0, [[1, 4], [1, 1]]))
        nc.gpsimd.collective_compute(
            "AllReduce", OP.add,
            replica_groups=[list(range(N_CORES))],
            ins=[cc_win.ap()], outs=[cc_wout.ap()])

        # one-time zero inits
        nc.vector.memset(_ap(PS, 0, [[4096, 128], [1, 4096]]), 0.0)
        nc.gpsimd.memset(zt.ap(), 0.0)
        nc.gpsimd.memset(dsh.ap(), 0.0)
        nc.gpsimd.memset(dodd.ap(), 0.0)
        nc.gpsimd.memset(avmx.ap(), 0.0)
        ztf = _ap(zt, 0, [[1024, 128], [1, 1024]])
        per = 128 * 1024
        # zero padded-xc image + both dp plane canvases (borders stay zero)
        for t, n in [(xcp_dram, 2 * XCP_N),
                     (dp_dram[0], K2 * DPN), (dp_dram[1], K2 * DPN)]:
            nfull = n // per
            for i in range(nfull):
                nc.gpsimd.dma_start(
                    _ap(t, i * per, [[1024, 128], [1, 1024]]), ztf)
            rem = n - nfull * per
            if rem:
                fr = rem // 1024
                off = nfull * per
                if fr:
                    nc.gpsimd.dma_start(
                        _ap(t, off, [[1024, fr], [1, 1024]]),
                        _ap(zt, 0, [[1024, fr], [1, 1024]]))
                tail = rem - fr * 1024
                if tail:
                    nc.gpsimd.dma_start(
                        _ap(t, off + fr * 1024, [[tail, 1], [1, tail]]),
                        _ap(zt, 0, [[tail, 1], [1, tail]]))

        out_ps = _ap(PS, 2560, [[4096, 16], [1, 1024]])

        with (
            tc.tile_pool(name="xp", bufs=3) as xp,
            tc.tile_pool(name="ep", bufs=3) as ep,
            tc.tile_pool(name="st", bufs=1) as stp,
        ):
            def emit_patches(b, lo=0, hi=8570):
                # contiguous per-partition runs: 28 dmas x 7 parts
                qs = [nc.sync, nc.scalar, nc.gpsimd] if b == 0 else \
                    [nc.sync, nc.gpsimd]
                i = 0
                for h in range(2):
                    for cch in range(2):
                        for ky in range(7):
                            q = qs[i % len(qs)]
                            i += 1
                            q.dma_start(
                                _ap(patch[h], (cch * 49 + 7 * ky) * PATN + lo,
                                    [[PATN, 7], [1, hi - lo]]),
                                _ap(xcp_dram,
                                    cch * XCP_N + (64 * h + ky) * PW + lo,
                                    [[1, 7], [1, hi - lo]]))

            def emit_windows(b, r0, r1, dq=None):
                # contiguous slab reads of dp: rows [r0,r1) full 136 width
                ln = (r1 - r0) * DPW
                for h in range(2):
                    q1 = dq or nc.sync
                    q2 = dq or nc.gpsimd
                    q1.dma_start(
                        _ap(dsh, 64 * h * WIN_N + r0 * DPW,
                            [[WIN_N, 49], [1, ln]]),
                        _ap(dp_dram[b], (64 * h + 3 + r0) * DPW,
                            [[DPN, 49], [1, ln]]))
                    q2.dma_start(
                        _ap(dodd, 64 * h * WIN_N + r0 * DPW,
                            [[WIN_N, 49], [1, ln]]),
                        _ap(dp_dram[b], (64 * h + 3 + r0) * DPW + 1,
                            [[DPN, 49], [1, ln]]))

            def phase_A_gen(b, skip_patches=False):
                VE = nc.vector
                for g in range(2):
                    for nl in range(8):
                        n = 8 * g + nl
                        xt = xp.tile([128, 2048], F32R, tag="xt")
                        for cb in range(2):
                            dma(
                                _ap(xt.tensor, xt.offset + cb * 1024,
                                    [[2048, 128], [1, 1024]]),
                                _ap(xs, b * C * HW + cb * 128 * HW + n * 1024,
                                    [[HW, 128], [1, 1024]]))
                        et = ep.tile([128, 2048], BF16, tag="et")
                        nc.scalar.activation(et[:, :], xt[:, :].bitcast(F32),
                                             AF.Exp, bias=cstsb.ap(),
                                             scale=LSE_T)
                        sfa = _ap(selA_f, 16 * n, [[256, 128], [1, 16]])
                        sba = _ap(selA_b, 16 * n, [[256, 128], [1, 16]])
                        for s in range(2):
                            for cb in range(2):
                                sl = slice(cb * 1024 + s * 512,
                                           cb * 1024 + (s + 1) * 512)
                                st_f = (nl == 0 and cb == 0)
                                sp_f = (nl == 7 and cb == 1)
                                nc.tensor.matmul(
                                    _ap(PS, 1536 + s * 512,
                                        [[4096, 16], [1, 512]]),
                                    sfa, xt[:, sl], start=st_f, stop=sp_f)
                                nc.tensor.matmul(
                                    _ap(PS, 32 * 4096 + 1536 + s * 512,
                                        [[4096, 16], [1, 512]]),
                                    sba, et[:, sl], start=st_f, stop=sp_f)
                        yield
                    # ---- group readout (full 16-row blocks; the other
                    # group's rows are stale/zero and never consumed) ----
                    nc.scalar.copy(
                        _ap(avmx, 0, [[1024, 16], [1, 1024]]),
                        _ap(PS, 1536, [[4096, 16], [1, 1024]]))
                    nc.gpsimd.dma_start(
                        _ap(xcp_dram, 3 * PW + 3 + g * 64 * PW,
                            [[8 * PW, 8], [PW, 8], [1, 128]]),
                        _ap(avmx, 8 * g * 1024, [[1024, 8], [1, 1024]]))
                    # lse: ln(S) = Eraw*ln2 - 127*ln2 + ln(M), M in [1,2)
                    lse_ps = _ap(PS, 32 * 4096 + 1536, [[4096, 16], [1, 1024]])
                    bits = lse_ps.bitcast(I32)
                    ef_i = stp.tile([48, 1024], I32, tag="efi")
                    VE.tensor_scalar(ef_i[32:48, :], bits, 23, None,
                                     OP.arith_shift_right)
                    mf = stp.tile([48, 1024], F32, tag="mf")
                    VE.tensor_scalar(mf[32:48, :].bitcast(I32),
                                     bits, 0x007FFFFF, 0x3F800000,
                                     OP.bitwise_and, OP.bitwise_or)
                    ef = stp.tile([48, 1024], F32, tag="lnst")
                    VE.tensor_copy(ef[32:48, :], ef_i[32:48, :])
                    lnm = stp.tile([48, 1024], F32, tag="efi")
                    nc.scalar.activation(lnm[32:48, :], mf[32:48, :],
                                         AF.Ln)
                    nc.scalar.activation(mf[32:48, :], lnm[32:48, :],
                                         AF.Copy,
                                         bias=(LSE_C - 127.0 * LN2) / LSE_T,
                                         scale=1.0 / LSE_T)
                    VE.scalar_tensor_tensor(
                        _ap(avmx, 32 * 1024, [[1024, 16], [1, 1024]]),
                        ef[32:48, :], LN2 / LSE_T, mf[32:48, :],
                        OP.mult, OP.add)
                    nc.gpsimd.dma_start(
                        _ap(xcp_dram, XCP_N + 3 * PW + 3 + g * 64 * PW,
                            [[8 * PW, 8], [PW, 8], [1, 128]]),
                        _ap(avmx, (32 + 8 * g) * 1024, [[1024, 8], [1, 1024]]))
                    if g == 1 and not skip_patches:
                        emit_patches(b)
                    yield
                    # ---- B half g: D planes ----
                    for nl in range(8):
                        n = 8 * g + nl
                        for s in range(2):
                            slot = (2 * nl + s) % 2
                            dvx = _ap(PS, 64 * 4096 + 1536 + slot * 512,
                                      [[4096, 49], [1, 512]])
                            nc.tensor.matmul(
                                dvx,
                                _ap(sd0, n * K2, [[16 * K2, 48], [1, K2]]),
                                _ap(avmx, s * 512, [[1024, 48], [1, 512]]),
                                start=True, stop=True)
                            dst = _ap(dsb, 64 * 8192 + nl * 1024 + s * 512,
                                      [[8192, 49], [1, 512]])
                            if b == 0:
                                nc.vector.tensor_copy(dst, dvx)
                            else:
                                nc.scalar.copy(dst, dvx)
                        if nl % 2 == 1:
                            yield
                    # scatter half g: per-ky dmas, kx shift linear in
                    # partition (stride DPN-1), 256B rows
                    for ky in range(7):
                        nc.gpsimd.dma_start(
                            _ap(dp_dram[b],
                                7 * ky * DPN + (7 - ky + 64 * g) * DPW + 7,
                                [[DPN - 1, 7], [DPW, 64], [1, 128]]),
                            _ap(dsb, (64 + 7 * ky) * 8192,
                                [[8192, 7], [128, 64], [1, 128]]))
                    yield

            def sle(r0, i):
                return _ap(dsh, (r0 + 1 + i) * DPW + 4,
                           [[WIN_N, NP], [DPW, 8], [1, 128]])

            def slo(r0, i, j):
                return _ap(dodd, (r0 + 1 + i) * DPW + 3 + j,
                           [[WIN_N, NP], [DPW, 8], [1, 128]])

            def phase_C(b, feed=None, after_chunk=None, in_chunk7=None):
                for n in range(8):
                    r0 = 8 * n
                    wq = wgt[4 * (n % 2): 4 * (n % 2) + 4]
                    for q in range(2):
                        for h in range(2):
                            for g in range(3):
                                nc.tensor.matmul(
                                    _ap(PS, 64 * h * 4096 + g * 512,
                                        [[4096, 49], [1, 512]]),
                                    _ap(wsb, g * 49, [[147, 98], [1, 49]]),
                                    _ap(patch[h], (8 * n + 4 * q) * PW,
                                        [[PATN, 98], [PW, 4], [1, 128]]),
                                    start=True, stop=True)
                        if n == 7 and q == 1 and in_chunk7 is not None:
                            in_chunk7()
                        npv = lambda lo: _ap(PS, lo, [[4096, NP], [1, 512]])
                        wvq = lambda t: _ap(t, q * 512, [[1024, NP], [1, 512]])
                        # bilinear weights fused on ACT: relu(+-(conv+bias))
                        nc.scalar.activation(wvq(wq[0]), npv(0), AF.Relu,
                                             bias=nbsb.ap()[:NP, 0:1],
                                             scale=-1.0)
                        nc.scalar.activation(wvq(wq[1]), npv(0), AF.Relu,
                                             bias=bsb.ap()[:NP, 0:1])
                        nc.scalar.activation(wvq(wq[2]), npv(512), AF.Relu,
                                             bias=nbsb.ap()[:NP, 1:2],
                                             scale=-1.0)
                        nc.scalar.activation(wvq(wq[3]), npv(512), AF.Relu,
                                             bias=bsb.ap()[:NP, 1:2])
                        nc.scalar.activation(wvq(mkb[n % 2]), npv(1024),
                                             AF.Sigmoid,
                                             bias=bsb.ap()[:NP, 2:3])
                    if feed is not None:
                        for _ in range(5 if n < 6 else 0):
                            next(feed, None)
                    wv = lambda t: _ap(t, 0, [[1024, NP], [1, 1024]])
                    v = nc.vector
                    wym, wyp, wxm, wxp = (wv(w) for w in wq)
                    t = [wv(x) for x in tb]
                    D00 = sle(r0, 0)
                    v.tensor_sub(t[0], sle(r0, -1), D00)
                    v.tensor_sub(t[1], sle(r0, 1), D00)
                    v.tensor_mul(t[2], wym, t[0])
                    v.tensor_mul(t[3], wyp, t[1])
                    v.tensor_add(t[4], t[2], t[3])
                    v.tensor_add(t[5], D00, t[4])              # G0
                    v.tensor_sub(t[0], slo(r0, -1, -1), slo(r0, 0, -1))
                    v.tensor_sub(t[1], slo(r0, 1, -1), slo(r0, 0, -1))
                    v.tensor_mul(t[2], wym, t[0])
                    v.tensor_mul(t[3], wyp, t[1])
                    v.tensor_add(t[6], t[2], t[3])
                    v.tensor_add(t[7], slo(r0, 0, -1), t[6])   # Gm
                    v.tensor_sub(t[0], slo(r0, -1, 1), slo(r0, 0, 1))
                    v.tensor_sub(t[1], slo(r0, 1, 1), slo(r0, 0, 1))
                    v.tensor_mul(t[2], wym, t[0])
                    v.tensor_mul(t[3], wyp, t[1])
                    v.tensor_add(t[6], t[2], t[3])
                    v.tensor_add(t[8], slo(r0, 0, 1), t[6])    # Gp
                    v.tensor_sub(t[0], t[7], t[5])
                    v.tensor_sub(t[1], t[8], t[5])
                    v.tensor_mul(t[2], wxm, t[0])
                    v.tensor_mul(t[3], wxp, t[1])
                    v.tensor_add(t[6], t[2], t[3])
                    v.tensor_add(t[7], t[5], t[6])             # val
                    v.tensor_mul(wv(bab[n % 2]), wv(mkb[n % 2]), t[7])
                    scf = _ap(selC, 16 * n, [[128, NP], [1, 16]])
                    for s in range(2):
                        nc.tensor.matmul(
                            _ap(PS, 2560 + s * 512,
                                [[4096, 16], [1, 512]]),
                            scf,
                            _ap(bab[n % 2], s * 512, [[1024, NP], [1, 512]]),
                            start=(n == 0), stop=(n == 7))
                    if after_chunk is not None:
                        after_chunk(n)
                # item BN partial sums
                ob_v = _ap(out_sb, b * 1024, [[2048, 16], [1, 1024]])
                nc.scalar.copy(ob_v, out_ps)
                dump = stp.tile([128, 1024], F32, tag="mf")
                nc.scalar.activation(dump[0:16, :], ob_v, AF.Identity,
                                     accum_out=_ap(accs, 2 * b,
                                                   [[4, 16], [1, 1]]))
                nc.scalar.activation(dump[0:16, :], ob_v, AF.Square,
                                     accum_out=_ap(accs, 2 * b + 1,
                                                   [[4, 16], [1, 1]]))

            # ================= schedule =================
            a0 = phase_A_gen(0)
            for _ in a0:
                pass
            emit_windows(0, 0, WIN_R)

            a1 = phase_A_gen(1, skip_patches=True)

            def after_chunk(n):
                if n == 4:
                    emit_patches(1, 0, 4288)
                elif n == 5:
                    emit_windows(1, 0, 33, dq=nc.gpsimd)
                elif n == 6:
                    emit_windows(1, 33, 56, dq=nc.gpsimd)
                elif n == 7:
                    emit_windows(1, 56, WIN_R, dq=nc.gpsimd)
                # C0 chunk 5 runs after the full a1 drain (5 pulls x 6
                # chunks = 30 >= 28 yields), so scatter h1 precedes band0

            phase_C(0, feed=a1, after_chunk=after_chunk,
                    in_chunk7=lambda: emit_patches(1, 4288, 8570))
            for _ in a1:
                pass
            phase_C(1)

            # ---------- BN ----------
            bn_ps = _ap(PS, 3584, [[4096, 1], [1, 4]])
            nc.tensor.matmul(bn_ps, o16.ap(), accs.ap(),
                             start=True, stop=True)
            bnl = _ap(bnt, 0, [[16, 1], [1, 4]])
            nc.scalar.copy(bnl, bn_ps)
            dma(cc_in.ap(), bnl)
            nc.gpsimd.collective_compute(
                "AllReduce", OP.add,
                replica_groups=[list(range(N_CORES))],
                ins=[cc_in.ap()], outs=[cc_out.ap()])
            bnr = _ap(bnt, 4, [[16, 1], [1, 4]])
            dma(bnr, cc_out.ap())
            v = nc.vector
            e = lambda i: _ap(bnt, 4 + i, [[16, 1], [1, 1]])
            t = lambda i: _ap(bnt, 8 + i, [[16, 1], [1, 1]])
            v.tensor_add(t(0), e(0), e(2))                  # sum
            v.tensor_add(t(1), e(1), e(3))                  # sumsq
            v.tensor_scalar_mul(t(2), t(0), 1.0 / N_TOTAL)  # mean
            v.tensor_scalar_mul(t(3), t(1), 1.0 / N_TOTAL)  # E[x^2]
            v.tensor_mul(t(4), t(2), t(2))
            v.tensor_sub(t(5), t(3), t(4))                  # var
            v.tensor_scalar_add(t(5), t(5), BN_EPS)
            v.reciprocal(t(6), t(5))
            nc.scalar.sqrt(t(7), t(6))                      # rstd
            v.tensor_mul(_ap(bnt, 2, [[16, 1], [1, 1]]), t(7),
                         gbs.ap()[:, 0:1])                  # scale @ [0,2]
            v.tensor_mul(t(4), t(2), _ap(bnt, 2, [[16, 1], [1, 1]]))
            v.tensor_sub(_ap(bnt, 3, [[16, 1], [1, 1]]),
                         gbs.ap()[:, 1:2], t(4))            # bias @ [0,3]
            sb2 = _ap(bnt, 2, [[16, 1], [1, 2]])
            bcp = _ap(PS, 3600, [[4096, 16], [1, 2]])
            nc.tensor.matmul(bcp, on16.ap(), sb2, start=True, stop=True)
            nc.scalar.copy(_ap(bnt, 8, [[16, 16], [1, 2]]), bcp)
            for b in range(BPC):
                yb = stp.tile([128, 1024], F32, tag=f"yb{b}")
                nc.scalar.activation(yb[0:16, :],
                                     _ap(out_sb, b * 1024,
                                         [[2048, 16], [1, 1024]]),
                                     AF.Sigmoid,
                                     bias=_ap(bnt, 9, [[16, 16], [1, 1]]),
                                     scale=_ap(bnt, 8, [[16, 16], [1, 1]]))
                dma(_ap(y_d, b * HW, [[1024, 8], [8192, 2], [1, 1024]]),
                    yb[0:16, :])

    nc.compile()
    return nc


_NC_CACHE = None


def _get_nc():
    global _NC_CACHE
    if _NC_CACHE is None:
        _NC_CACHE = build_program()
    return _NC_CACHE


def make_host_constants(w_off, b_off, w_dcn, gamma, beta):
    bf = ml_dtypes.bfloat16
    orig = np.empty(147, np.int64)
    for g in range(3):
        for kk in range(49):
            orig[g * 49 + kk] = (2 * kk, 2 * kk + 1, 98 + kk)[g]
    wof = w_off.reshape(147, 2, 7, 7)
    wcl = np.zeros((98, 147), np.float32)
    for c in range(2):
        for ky in range(7):
            for kx in range(7):
                wcl[c * 49 + 7 * ky + kx, :] = wof[orig, c, ky, kx]
    # bias over partition convention p = 64*half + k  (holes zero)
    bias_t = np.zeros((128, 3), np.float32)
    for g in range(3):
        bg = b_off[orig[g * 49:(g + 1) * 49]]
        bias_t[0:49, g] = bg
        bias_t[64:113, g] = bg
    dcn = w_dcn.reshape(2, 49).astype(np.float32)
    sd0 = np.zeros((48, 16 * K2), np.float32)
    for n in range(16):
        sd0[n, 49 * n:49 * (n + 1)] = dcn[0]
        sd0[32 + n, 49 * n:49 * (n + 1)] = dcn[1]
    # phase-A row-spread selectors [128, 16*16]
    selA_f = np.zeros((128, 256), np.float32)
    selA_b = np.zeros((128, 256), np.float32)
    for n in range(16):
        selA_f[:, 16 * n + n] = 1.0 / C
        selA_b[:, 16 * n + n] = 1.0
    # phase-C sum-over-k selectors [128, 16*8]
    selC = np.zeros((128, 128), np.float32)
    for n in range(8):
        selC[0:49, 16 * n + 2 * n] = 1.0
        selC[64:113, 16 * n + 2 * n + 1] = 1.0
    return {
        "wc": wcl.astype(np.float16),
        "bias": bias_t,
        "sd0": sd0.astype(np.float16),
        "selA_f": selA_f,
        "selA_b": selA_b.astype(bf),
        "selC": selC.astype(np.float16),
        "o16": np.ones((16, 1), np.float32),
        "on16": np.ones((1, 16), np.float32),
        "gb": np.array([[float(np.reshape(gamma, -1)[0]),
                         float(np.reshape(beta, -1)[0])]], np.float32),
        "cst": np.full((128, 1), -LSE_C, np.float32),
    }


def make_in_maps(x, w_off, b_off, w_dcn, gamma, beta):
    consts = make_host_constants(w_off, b_off, w_dcn, gamma, beta)
    in_maps = []
    for i in range(N_CORES):
        m = dict(consts)
        m["xs"] = np.ascontiguousarray(
            x[i * BPC:(i + 1) * BPC].reshape(BPC, C, HW).astype(np.float32))
        in_maps.append(m)
    return in_maps


def kernel(x, w_off, b_off, w_dcn, gamma, beta):
    x = np.asarray(x, np.float32)
    nc = _get_nc()
    in_maps = make_in_maps(x, np.asarray(w_off, np.float32),
                           np.asarray(b_off, np.float32),
                           np.asarray(w_dcn, np.float32),
                           np.asarray(gamma, np.float32),
                           np.asarray(beta, np.float32))
    trace = bool(int(os.environ.get("KERNEL_TRACE", "0")))
    res = run_bass_kernel_spmd(nc, in_maps, core_ids=list(range(N_CORES)),
                               trace=trace)
    ys = [np.asarray(res.results[i]["y"], np.float32).reshape(BPC, HW)
          for i in range(N_CORES)]
    out = np.stack(ys).reshape(B, 1, H, W)
    kernel.last_exec_time_ns = res.exec_time_ns
    return out


# revision 26
# speedup vs baseline: 1.1300x; 1.0267x over previous
"""Trainium2 Bass kernel for nn_DC_SpatialAttention (deformable-conv spatial attention).

Sharding: pure data-parallel over batch, 2 batch items per NeuronCore x 8 cores.
v2 pipeline (per batch item):
  A: stream x [256,16384] f32 in 16 chunks (2 HWDGE queues); per chunk
     mean row (PE f32r) + exp (ACT) + exp-sum row (PE bf16) into per-half
     group psum regions; per-half readout (ln-split chain) -> avmx + xcp.
  B (per half, pipelined right after its A group): D_k = dcn*(avg,mx)
     via PE -> psum -> dsb (bf16) -> ONE 4-level-AP scatter DMA into the
     pre-shifted plane canvas dp[item] (DRAM, 136x136 per k).
  Windows: dsh/dodd = contiguous 66*136-row slabs of dp (1 DMA per
     half/parity; rows wrap, all tent reads 4B-aligned).
  Patches: per (h,c,ky) contiguous 8570-elem runs of xcp -> conv matmuls
     use 3-level rhs APs (no im2col copy).
  C: offset conv (12 matmuls/chunk, FD=512) -> ACT relu/sigmoid FD=1024
     -> 25-op all-bf16 DVE tent -> masked val -> PE sum over k (bf16).
Item-1 A/B is generator-fed inside C0; item-1 windows reload in 3 row
bands and patches reload after C0's last conv (WAR-safe, tiny stall).
BatchNorm: per-core sums -> AllReduce -> affine+sigmoid -> y.

PSUM map (f32 cols x partitions):
  conv   parts 0-112, cols 0:3072   (g*1024+q*512; g=oy,ox,mask)
  mean   parts 0-15  (8g..), cols 3072:4096 (s*512)
  out    parts 16-31, cols 3072:4096
  lse    parts 32-47 (32+8g..), cols 3072:4096
  dvx    parts 64-112, slots 3072:3584 / 3584:4096
"""

import os
import numpy as np
import ml_dtypes

import concourse.bass as bass
import concourse.bacc as bacc
import concourse.mybir as mybir
import concourse.tile as tile
from concourse.bass_utils import run_bass_kernel_spmd

F32 = mybir.dt.float32
F32R = mybir.dt.float32r
BF16 = mybir.dt.bfloat16
F16 = mybir.dt.float16
I32 = mybir.dt.int32
AF = mybir.ActivationFunctionType
OP = mybir.AluOpType

# ---------------- problem constants (hardcoded) ----------------
B, C, H, W = 16, 256, 128, 128
HW = H * W
K2 = 49
BN_EPS = 1e-5
N_CORES = 8
BPC = B // N_CORES

LSE_T = 45.0
LSE_C = 153.0
LN2 = 0.6931471805599453

PW = H + 6                      # 134 padded xc width
XCP_N = PW * PW                 # 17956
DPW = H + 8                     # 136 plane width
DPN = DPW * DPW                 # 18496
WIN_R = 66
WIN_N = WIN_R * DPW             # 8976
PATN = 64 * PW                  # 8576 patch free width
NP = 113                        # tent partitions 0:49 + 64:113 (holes 49:64)

N_TOTAL = float(B * HW)


def _ap(t, off, pairs):
    return bass.AP(t, off, [list(p) for p in pairs])


def build_program(debug=False):
    nc = bacc.Bacc("TRN2", target_bir_lowering=False, debug=False,
                   num_devices=N_CORES)

    xs = nc.dram_tensor("xs", [BPC, C, HW], F32R, kind="ExternalInput")
    wc = nc.dram_tensor("wc", [98, 147], F16, kind="ExternalInput")
    bias_d = nc.dram_tensor("bias", [128, 3], F32, kind="ExternalInput")
    sd0_d = nc.dram_tensor("sd0", [48, 16 * K2], F16, kind="ExternalInput")
    selA_f_d = nc.dram_tensor("selA_f", [128, 256], F32R, kind="ExternalInput")
    selA_b_d = nc.dram_tensor("selA_b", [128, 256], BF16, kind="ExternalInput")
    selC_d = nc.dram_tensor("selC", [128, 128], F16, kind="ExternalInput")
    o16_d = nc.dram_tensor("o16", [16, 1], F32, kind="ExternalInput")
    on16_d = nc.dram_tensor("on16", [1, 16], F32, kind="ExternalInput")
    gb_d = nc.dram_tensor("gb", [1, 2], F32, kind="ExternalInput")
    cst_d = nc.dram_tensor("cst", [128, 1], F32, kind="ExternalInput")
    y_d = nc.dram_tensor("y", [BPC, HW], F32, kind="ExternalOutput")

    dp_dram = [nc.dram_tensor(f"dp_dram{i}", [K2 * DPN], F16)
               for i in range(2)]
    xcp_dram = nc.dram_tensor("xcp_dram", [2 * XCP_N], F16)
    cc_in = nc.dram_tensor("cc_in", [4], F32)
    cc_out = nc.dram_tensor("cc_out", [4], F32, addr_space="Shared")
    cc_win = nc.dram_tensor("cc_win", [4], F32)
    cc_wout = nc.dram_tensor("cc_wout", [4], F32, addr_space="Shared")

    PS = nc.alloc_psum_tensor("PS", [128, 4096], F32)

    with tile.TileContext(nc) as tc:
        dsh = nc.alloc_sbuf_tensor("dsh", [128, WIN_N], F16)
        dodd = nc.alloc_sbuf_tensor("dodd", [128, WIN_N], F16)
        dsb = nc.alloc_sbuf_tensor("dsb", [128, 8192], F16)
        patch = [nc.alloc_sbuf_tensor(f"patch{h}", [98, PATN], F16)
                 for h in range(2)]
        avmx = nc.alloc_sbuf_tensor("avmx", [48, 1024], F16)
        out_sb = nc.alloc_sbuf_tensor("out_sb", [16, 2 * 1024], F32)
        accs = nc.alloc_sbuf_tensor("accs", [16, 4], F32)
        bnt = nc.alloc_sbuf_tensor("bnt", [16, 16], F32)
        wsb = nc.alloc_sbuf_tensor("wsb", [98, 147], F16)
        bsb = nc.alloc_sbuf_tensor("bsb", [128, 3], F32)
        nbsb = nc.alloc_sbuf_tensor("nbsb", [128, 3], F32)
        sd0 = nc.alloc_sbuf_tensor("sd0_s", [48, 16 * K2], F16)
        selA_f = nc.alloc_sbuf_tensor("selA_f_s", [128, 256], F32R)
        selA_b = nc.alloc_sbuf_tensor("selA_b_s", [128, 256], BF16)
        selC = nc.alloc_sbuf_tensor("selC_s", [128, 128], F16)
        o16 = nc.alloc_sbuf_tensor("o16_s", [16, 1], F32)
        on16 = nc.alloc_sbuf_tensor("on16_s", [1, 16], F32)
        gbs = nc.alloc_sbuf_tensor("gbs", [1, 2], F32)
        cstsb = nc.alloc_sbuf_tensor("cst_s", [128, 1], F32)
        zt = nc.alloc_sbuf_tensor("zt", [128, 1024], F16)
        tb = [nc.alloc_sbuf_tensor(f"tb{i}", [128, 1024], F16)
              for i in range(9)]
        wgt = [nc.alloc_sbuf_tensor(f"wgt{i}", [128, 1024], F16)
               for i in range(8)]
        mkb = [nc.alloc_sbuf_tensor(f"mk{i}", [128, 1024], F16)
               for i in range(2)]
        bab = [nc.alloc_sbuf_tensor(f"ba{i}", [128, 1024], F16)
               for i in range(2)]

        dma = nc.sync.dma_start

        dma(wsb.ap(), wc.ap())
        dma(bsb.ap(), bias_d.ap())
        dma(sd0.ap(), sd0_d.ap())
        dma(selA_f.ap(), selA_f_d.ap())
        dma(selA_b.ap(), selA_b_d.ap())
        dma(selC.ap(), selC_d.ap())
        dma(o16.ap(), o16_d.ap())
        dma(on16.ap(), on16_d.ap())
        dma(gbs.ap(), gb_d.ap())
        dma(cstsb.ap(), cst_d.ap())
        nc.vector.tensor_scalar_mul(nbsb.ap(), bsb.ap(), -1.0)
        # warm up the collective path while A0 streams
        dma(cc_win.ap(), _ap(cstsb, 0, [[1, 4], [1, 1]]))
        nc.gpsimd.collective_compute(
            "AllReduce", OP.add,
            replica_groups=[list(range(N_CORES))],
            ins=[cc_win.ap()], outs=[cc_wout.ap()])

        # one-time zero inits
        nc.vector.memset(_ap(PS, 0, [[4096, 128], [1, 4096]]), 0.0)
        nc.gpsimd.memset(zt.ap(), 0.0)
        nc.gpsimd.memset(dsh.ap(), 0.0)
        nc.gpsimd.memset(dodd.ap(), 0.0)
        nc.gpsimd.memset(avmx.ap(), 0.0)
        ztf = _ap(zt, 0, [[1024, 128], [1, 1024]])
        per = 128 * 1024
        # zero padded-xc image + both dp plane canvases (borders stay zero)
        for t, n in [(xcp_dram, 2 * XCP_N),
                     (dp_dram[0], K2 * DPN), (dp_dram[1], K2 * DPN)]:
            nfull = n // per
            for i in range(nfull):
                nc.gpsimd.dma_start(
                    _ap(t, i * per, [[1024, 128], [1, 1024]]), ztf)
            rem = n - nfull * per
            if rem:
                fr = rem // 1024
                off = nfull * per
                if fr:
                    nc.gpsimd.dma_start(
                        _ap(t, off, [[1024, fr], [1, 1024]]),
                        _ap(zt, 0, [[1024, fr], [1, 1024]]))
                tail = rem - fr * 1024
                if tail:
                    nc.gpsimd.dma_start(
                        _ap(t, off + fr * 1024, [[tail, 1], [1, tail]]),
                        _ap(zt, 0, [[tail, 1], [1, tail]]))

        out_ps = _ap(PS, 2560, [[4096, 16], [1, 1024]])

        with (
            tc.tile_pool(name="xp", bufs=3) as xp,
            tc.tile_pool(name="ep", bufs=3) as ep,
            tc.tile_pool(name="st", bufs=1) as stp,
        ):
            def emit_patches(b, lo=0, hi=8570):
                # contiguous per-partition runs: 28 dmas x 7 parts
                qs = [nc.sync, nc.scalar, nc.gpsimd] if b == 0 else \
                    [nc.sync, nc.gpsimd]
                i = 0
                for h in range(2):
                    for cch in range(2):
                        for ky in range(7):
                            q = qs[i % len(qs)]
                            i += 1
                            q.dma_start(
                                _ap(patch[h], (cch * 49 + 7 * ky) * PATN + lo,
                                    [[PATN, 7], [1, hi - lo]]),
                                _ap(xcp_dram,
                                    cch * XCP_N + (64 * h + ky) * PW + lo,
                                    [[1, 7], [1, hi - lo]]))

            def emit_windows(b, r0, r1, dq=None):
                # contiguous slab reads of dp: rows [r0,r1) full 136 width
                ln = (r1 - r0) * DPW
                for h in range(2):
                    q1 = dq or nc.sync
                    q2 = dq or nc.gpsimd
                    q1.dma_start(
                        _ap(dsh, 64 * h * WIN_N + r0 * DPW,
                            [[WIN_N, 49], [1, ln]]),
                        _ap(dp_dram[b], (64 * h + 3 + r0) * DPW,
                            [[DPN, 49], [1, ln]]))
                    q2.dma_start(
                        _ap(dodd, 64 * h * WIN_N + r0 * DPW,
                            [[WIN_N, 49], [1, ln]]),
                        _ap(dp_dram[b], (64 * h + 3 + r0) * DPW + 1,
                            [[DPN, 49], [1, ln]]))

            def phase_A_gen(b, skip_patches=False):
                VE = nc.vector
                for g in range(2):
                    for nl in range(8):
                        n = 8 * g + nl
                        xt = xp.tile([128, 2048], F32R, tag="xt")
                        for cb in range(2):
                            dma(
                                _ap(xt.tensor, xt.offset + cb * 1024,
                                    [[2048, 128], [1, 1024]]),
                                _ap(xs, b * C * HW + cb * 128 * HW + n * 1024,
                                    [[HW, 128], [1, 1024]]))
                        et = ep.tile([128, 2048], BF16, tag="et")
                        # exp split per cb half so lse mms start earlier
                        for cb in range(2):
                            nc.scalar.activation(
                                et[:, cb * 1024:(cb + 1) * 1024],
                                xt[:, cb * 1024:(cb + 1) * 1024].bitcast(F32),
                                AF.Exp, bias=cstsb.ap(), scale=LSE_T)
                        sfa = _ap(selA_f, 16 * n, [[256, 128], [1, 16]])
                        sba = _ap(selA_b, 16 * n, [[256, 128], [1, 16]])
                        for cb in range(2):
                            for s in range(2):
                                sl = slice(cb * 1024 + s * 512,
                                           cb * 1024 + (s + 1) * 512)
                                nc.tensor.matmul(
                                    _ap(PS, 1536 + s * 512,
                                        [[4096, 16], [1, 512]]),
                                    sfa, xt[:, sl],
                                    start=(nl == 0 and cb == 0),
                                    stop=(nl == 7 and cb == 1))
                        for cb in range(2):
                            for s in range(2):
                                sl = slice(cb * 1024 + s * 512,
                                           cb * 1024 + (s + 1) * 512)
                                nc.tensor.matmul(
                                    _ap(PS, 32 * 4096 + 1536 + s * 512,
                                        [[4096, 16], [1, 512]]),
                                    sba, et[:, sl],
                                    start=(nl == 0 and cb == 0),
                                    stop=(nl == 7 and cb == 1))
                        yield
                    # ---- group readout (full 16-row blocks; the other
                    # group's rows are stale/zero and never consumed) ----
                    nc.scalar.copy(
                        _ap(avmx, 0, [[1024, 16], [1, 1024]]),
                        _ap(PS, 1536, [[4096, 16], [1, 1024]]))
                    nc.gpsimd.dma_start(
                        _ap(xcp_dram, 3 * PW + 3 + g * 64 * PW,
                            [[8 * PW, 8], [PW, 8], [1, 128]]),
                        _ap(avmx, 8 * g * 1024, [[1024, 8], [1, 1024]]))
                    # lse: ln(S) = Eraw*ln2 - 127*ln2 + ln(M), M in [1,2)
                    lse_ps = _ap(PS, 32 * 4096 + 1536, [[4096, 16], [1, 1024]])
                    bits = lse_ps.bitcast(I32)
                    ef_i = stp.tile([48, 1024], I32, tag="efi")
                    VE.tensor_scalar(ef_i[32:48, :], bits, 23, None,
                                     OP.arith_shift_right)
                    mf = stp.tile([48, 1024], F32, tag="mf")
                    VE.tensor_scalar(mf[32:48, :].bitcast(I32),
                                     bits, 0x007FFFFF, 0x3F800000,
                                     OP.bitwise_and, OP.bitwise_or)
                    ef = stp.tile([48, 1024], F32, tag="lnst")
                    VE.tensor_copy(ef[32:48, :], ef_i[32:48, :])
                    lnm = stp.tile([48, 1024], F32, tag="efi")
                    nc.scalar.activation(lnm[32:48, :], mf[32:48, :],
                                         AF.Ln)
                    nc.scalar.activation(mf[32:48, :], lnm[32:48, :],
                                         AF.Copy,
                                         bias=(LSE_C - 127.0 * LN2) / LSE_T,
                                         scale=1.0 / LSE_T)
                    VE.scalar_tensor_tensor(
                        _ap(avmx, 32 * 1024, [[1024, 16], [1, 1024]]),
                        ef[32:48, :], LN2 / LSE_T, mf[32:48, :],
                        OP.mult, OP.add)
                    nc.gpsimd.dma_start(
                        _ap(xcp_dram, XCP_N + 3 * PW + 3 + g * 64 * PW,
                            [[8 * PW, 8], [PW, 8], [1, 128]]),
                        _ap(avmx, (32 + 8 * g) * 1024, [[1024, 8], [1, 1024]]))
                    if g == 1 and not skip_patches:
                        emit_patches(b)
                    yield
                    # ---- B half g: D planes ----
                    for nl in range(8):
                        n = 8 * g + nl
                        for s in range(2):
                            slot = (2 * nl + s) % 2
                            dvx = _ap(PS, 64 * 4096 + 1536 + slot * 512,
                                      [[4096, 49], [1, 512]])
                            nc.tensor.matmul(
                                dvx,
                                _ap(sd0, n * K2, [[16 * K2, 48], [1, K2]]),
                                _ap(avmx, s * 512, [[1024, 48], [1, 512]]),
                                start=True, stop=True)
                            dst = _ap(dsb, 64 * 8192 + nl * 1024 + s * 512,
                                      [[8192, 49], [1, 512]])
                            if b == 0:
                                nc.vector.tensor_copy(dst, dvx)
                            else:
                                nc.scalar.copy(dst, dvx)
                        if nl % 2 == 1:
                            yield
                    # scatter half g: per-ky dmas, kx shift linear in
                    # partition (stride DPN-1), 256B rows
                    for ky in range(7):
                        nc.gpsimd.dma_start(
                            _ap(dp_dram[b],
                                7 * ky * DPN + (7 - ky + 64 * g) * DPW + 7,
                                [[DPN - 1, 7], [DPW, 64], [1, 128]]),
                            _ap(dsb, (64 + 7 * ky) * 8192,
                                [[8192, 7], [128, 64], [1, 128]]))
                    yield

            def sle(r0, i):
                return _ap(dsh, (r0 + 1 + i) * DPW + 4,
                           [[WIN_N, NP], [DPW, 8], [1, 128]])

            def slo(r0, i, j):
                return _ap(dodd, (r0 + 1 + i) * DPW + 3 + j,
                           [[WIN_N, NP], [DPW, 8], [1, 128]])

            def phase_C(b, feed=None, after_chunk=None, in_chunk7=None):
                for n in range(8):
                    r0 = 8 * n
                    wq = wgt[4 * (n % 2): 4 * (n % 2) + 4]
                    for q in range(2):
                        for h in range(2):
                            for g in range(3):
                                nc.tensor.matmul(
                                    _ap(PS, 64 * h * 4096 + g * 512,
                                        [[4096, 49], [1, 512]]),
                                    _ap(wsb, g * 49, [[147, 98], [1, 49]]),
                                    _ap(patch[h], (8 * n + 4 * q) * PW,
                                        [[PATN, 98], [PW, 4], [1, 128]]),
                                    start=True, stop=True)
                        if n == 7 and q == 1 and in_chunk7 is not None:
                            in_chunk7()
                        npv = lambda lo: _ap(PS, lo, [[4096, NP], [1, 512]])
                        wvq = lambda t: _ap(t, q * 512, [[1024, NP], [1, 512]])
                        # bilinear weights fused on ACT: relu(+-(conv+bias))
                        nc.scalar.activation(wvq(wq[0]), npv(0), AF.Relu,
                                             bias=nbsb.ap()[:NP, 0:1],
                                             scale=-1.0)
                        nc.scalar.activation(wvq(wq[1]), npv(0), AF.Relu,
                                             bias=bsb.ap()[:NP, 0:1])
                        nc.scalar.activation(wvq(wq[2]), npv(512), AF.Relu,
                                             bias=nbsb.ap()[:NP, 1:2],
                                             scale=-1.0)
                        nc.scalar.activation(wvq(wq[3]), npv(512), AF.Relu,
                                             bias=bsb.ap()[:NP, 1:2])
                        nc.scalar.activation(wvq(mkb[n % 2]), npv(1024),
                                             AF.Sigmoid,
                                             bias=bsb.ap()[:NP, 2:3])
                    if feed is not None:
                        for _ in range(5 if n < 6 else 0):
                            next(feed, None)
                    wv = lambda t: _ap(t, 0, [[1024, NP], [1, 1024]])
                    v = nc.vector
                    wym, wyp, wxm, wxp = (wv(w) for w in wq)
                    t = [wv(x) for x in tb]
                    D00 = sle(r0, 0)
                    v.tensor_sub(t[0], sle(r0, -1), D00)
                    v.tensor_sub(t[1], sle(r0, 1), D00)
                    v.tensor_mul(t[2], wym, t[0])
                    v.tensor_mul(t[3], wyp, t[1])
                    v.tensor_add(t[4], t[2], t[3])
                    v.tensor_add(t[5], D00, t[4])              # G0
                    v.tensor_sub(t[0], slo(r0, -1, -1), slo(r0, 0, -1))
                    v.tensor_sub(t[1], slo(r0, 1, -1), slo(r0, 0, -1))
                    v.tensor_mul(t[2], wym, t[0])
                    v.tensor_mul(t[3], wyp, t[1])
                    v.tensor_add(t[6], t[2], t[3])
                    v.tensor_add(t[7], slo(r0, 0, -1), t[6])   # Gm
                    v.tensor_sub(t[0], slo(r0, -1, 1), slo(r0, 0, 1))
                    v.tensor_sub(t[1], slo(r0, 1, 1), slo(r0, 0, 1))
                    v.tensor_mul(t[2], wym, t[0])
                    v.tensor_mul(t[3], wyp, t[1])
                    v.tensor_add(t[6], t[2], t[3])
                    v.tensor_add(t[8], slo(r0, 0, 1), t[6])    # Gp
                    v.tensor_sub(t[0], t[7], t[5])
                    v.tensor_sub(t[1], t[8], t[5])
                    v.tensor_mul(t[2], wxm, t[0])
                    v.tensor_mul(t[3], wxp, t[1])
                    v.tensor_add(t[6], t[2], t[3])
                    v.tensor_add(t[7], t[5], t[6])             # val
                    v.tensor_mul(wv(bab[n % 2]), wv(mkb[n % 2]), t[7])
                    scf = _ap(selC, 16 * n, [[128, NP], [1, 16]])
                    for s in range(2):
                        nc.tensor.matmul(
                            _ap(PS, 2560 + s * 512,
                                [[4096, 16], [1, 512]]),
                            scf,
                            _ap(bab[n % 2], s * 512, [[1024, NP], [1, 512]]),
                            start=(n == 0), stop=(n == 7))
                    if after_chunk is not None:
                        after_chunk(n)
                # item BN partial sums
                ob_v = _ap(out_sb, b * 1024, [[2048, 16], [1, 1024]])
                nc.scalar.copy(ob_v, out_ps)
                dump = stp.tile([128, 1024], F32, tag="mf")
                nc.scalar.activation(dump[0:16, :], ob_v, AF.Identity,
                                     accum_out=_ap(accs, 2 * b,
                                                   [[4, 16], [1, 1]]))
                nc.scalar.activation(dump[0:16, :], ob_v, AF.Square,
                                     accum_out=_ap(accs, 2 * b + 1,
                                                   [[4, 16], [1, 1]]))

            # ================= schedule =================
            a0 = phase_A_gen(0)
            for _ in a0:
                pass
            emit_windows(0, 0, WIN_R)

            a1 = phase_A_gen(1, skip_patches=True)

            def after_chunk(n):
                if n == 4:
                    emit_patches(1, 0, 4288)
                elif n == 5:
                    emit_windows(1, 0, 33, dq=nc.gpsimd)
                elif n == 6:
                    emit_windows(1, 33, 56, dq=nc.gpsimd)
                elif n == 7:
                    emit_windows(1, 56, WIN_R, dq=nc.gpsimd)
                # C0 chunk 5 runs after the full a1 drain (5 pulls x 6
                # chunks = 30 >= 28 yields), so scatter h1 precedes band0

            phase_C(0, feed=a1, after_chunk=after_chunk,
                    in_chunk7=lambda: emit_patches(1, 4288, 8570))
            for _ in a1:
                pass
            phase_C(1)

            # ---------- BN ----------
            bn_ps = _ap(PS, 3584, [[4096, 1], [1, 4]])
            nc.tensor.matmul(bn_ps, o16.ap(), accs.ap(),
                             start=True, stop=True)
            bnl = _ap(bnt, 0, [[16, 1], [1, 4]])
            nc.scalar.copy(bnl, bn_ps)
            dma(cc_in.ap(), bnl)
            nc.gpsimd.collective_compute(
                "AllReduce", OP.add,
                replica_groups=[list(range(N_CORES))],
                ins=[cc_in.ap()], outs=[cc_out.ap()])
            bnr = _ap(bnt, 4, [[16, 1], [1, 4]])
            dma(bnr, cc_out.ap())
            v = nc.vector
            e = lambda i: _ap(bnt, 4 + i, [[16, 1], [1, 1]])
            t = lambda i: _ap(bnt, 8 + i, [[16, 1], [1, 1]])
            v.tensor_add(t(0), e(0), e(2))                  # sum
            v.tensor_add(t(1), e(1), e(3))                  # sumsq
            v.tensor_scalar_mul(t(2), t(0), 1.0 / N_TOTAL)  # mean
            v.tensor_scalar_mul(t(3), t(1), 1.0 / N_TOTAL)  # E[x^2]
            v.tensor_mul(t(4), t(2), t(2))
            v.tensor_sub(t(5), t(3), t(4))                  # var
            v.tensor_scalar_add(t(5), t(5), BN_EPS)
            v.reciprocal(t(6), t(5))
            nc.scalar.sqrt(t(7), t(6))                      # rstd
            v.tensor_mul(_ap(bnt, 2, [[16, 1], [1, 1]]), t(7),
                         gbs.ap()[:, 0:1])                  # scale @ [0,2]
            v.tensor_mul(t(4), t(2), _ap(bnt, 2, [[16, 1], [1, 1]]))
            v.tensor_sub(_ap(bnt, 3, [[16, 1], [1, 1]]),
                         gbs.ap()[:, 1:2], t(4))            # bias @ [0,3]
            sb2 = _ap(bnt, 2, [[16, 1], [1, 2]])
            bcp = _ap(PS, 3600, [[4096, 16], [1, 2]])
            nc.tensor.matmul(bcp, on16.ap(), sb2, start=True, stop=True)
            nc.scalar.copy(_ap(bnt, 8, [[16, 16], [1, 2]]), bcp)
            for b in range(BPC):
                yb = stp.tile([128, 1024], F32, tag=f"yb{b}")
                nc.scalar.activation(yb[0:16, :],
                                     _ap(out_sb, b * 1024,
                                         [[2048, 16], [1, 1024]]),
                                     AF.Sigmoid,
                                     bias=_ap(bnt, 9, [[16, 16], [1, 1]]),
                                     scale=_ap(bnt, 8, [[16, 16], [1, 1]]))
                dma(_ap(y_d, b * HW, [[1024, 8], [8192, 2], [1, 1024]]),
                    yb[0:16, :])

    nc.compile()
    return nc


_NC_CACHE = None


def _get_nc():
    global _NC_CACHE
    if _NC_CACHE is None:
        _NC_CACHE = build_program()
    return _NC_CACHE


def make_host_constants(w_off, b_off, w_dcn, gamma, beta):
    bf = ml_dtypes.bfloat16
    orig = np.empty(147, np.int64)
    for g in range(3):
        for kk in range(49):
            orig[g * 49 + kk] = (2 * kk, 2 * kk + 1, 98 + kk)[g]
    wof = w_off.reshape(147, 2, 7, 7)
    wcl = np.zeros((98, 147), np.float32)
    for c in range(2):
        for ky in range(7):
            for kx in range(7):
                wcl[c * 49 + 7 * ky + kx, :] = wof[orig, c, ky, kx]
    # bias over partition convention p = 64*half + k  (holes zero)
    bias_t = np.zeros((128, 3), np.float32)
    for g in range(3):
        bg = b_off[orig[g * 49:(g + 1) * 49]]
        bias_t[0:49, g] = bg
        bias_t[64:113, g] = bg
    dcn = w_dcn.reshape(2, 49).astype(np.float32)
    sd0 = np.zeros((48, 16 * K2), np.float32)
    for n in range(16):
        sd0[n, 49 * n:49 * (n + 1)] = dcn[0]
        sd0[32 + n, 49 * n:49 * (n + 1)] = dcn[1]
    # phase-A row-spread selectors [128, 16*16]
    selA_f = np.zeros((128, 256), np.float32)
    selA_b = np.zeros((128, 256), np.float32)
    for n in range(16):
        selA_f[:, 16 * n + n] = 1.0 / C
        selA_b[:, 16 * n + n] = 1.0
    # phase-C sum-over-k selectors [128, 16*8]
    selC = np.zeros((128, 128), np.float32)
    for n in range(8):
        selC[0:49, 16 * n + 2 * n] = 1.0
        selC[64:113, 16 * n + 2 * n + 1] = 1.0
    return {
        "wc": wcl.astype(np.float16),
        "bias": bias_t,
        "sd0": sd0.astype(np.float16),
        "selA_f": selA_f,
        "selA_b": selA_b.astype(bf),
        "selC": selC.astype(np.float16),
        "o16": np.ones((16, 1), np.float32),
        "on16": np.ones((1, 16), np.float32),
        "gb": np.array([[float(np.reshape(gamma, -1)[0]),
                         float(np.reshape(beta, -1)[0])]], np.float32),
        "cst": np.full((128, 1), -LSE_C, np.float32),
    }


def make_in_maps(x, w_off, b_off, w_dcn, gamma, beta):
    consts = make_host_constants(w_off, b_off, w_dcn, gamma, beta)
    in_maps = []
    for i in range(N_CORES):
        m = dict(consts)
        m["xs"] = np.ascontiguousarray(
            x[i * BPC:(i + 1) * BPC].reshape(BPC, C, HW).astype(np.float32))
        in_maps.append(m)
    return in_maps


def kernel(x, w_off, b_off, w_dcn, gamma, beta):
    x = np.asarray(x, np.float32)
    nc = _get_nc()
    in_maps = make_in_maps(x, np.asarray(w_off, np.float32),
                           np.asarray(b_off, np.float32),
                           np.asarray(w_dcn, np.float32),
                           np.asarray(gamma, np.float32),
                           np.asarray(beta, np.float32))
    trace = bool(int(os.environ.get("KERNEL_TRACE", "0")))
    res = run_bass_kernel_spmd(nc, in_maps, core_ids=list(range(N_CORES)),
                               trace=trace)
    ys = [np.asarray(res.results[i]["y"], np.float32).reshape(BPC, HW)
          for i in range(N_CORES)]
    out = np.stack(ys).reshape(B, 1, H, W)
    kernel.last_exec_time_ns = res.exec_time_ns
    return out
